# revision 8
# baseline (speedup 1.0000x reference)
"""Causal self-attention (B=2, T=2048, D=1024, H=16) on 8 trn2 cores.

Sharding: tensor-parallel over heads x data-parallel over batch.
Core c handles batch b = c // 4, head group g = c % 4 (heads 4g..4g+3).
Host pre-slices/pre-transposes weight+activation shards (cast to bf16);
each core returns a partial y (its heads' contribution); host sums
groups of 4.

Kernel structure (v2): pipelined per t-tile of 512 —
  q-proj(tt) -> attention on old k/v s-chunks -> k/v-proj(tt) ->
  diagonal-quad attention -> on-chip softmax norm -> out-proj(tt).
All matmuls bf16 (f32 psum). Softmax sums come from a ones-column in V;
normalization is reciprocal (DVE) + partition_broadcast (Pool) + mul,
no DRAM roundtrip. Upper half of each diagonal quad is computed at half
t-width.
"""

import os
import sys

for _p in ("/opt/trn_rl_repo", "/root/.axon_site/_ro/trn_rl_repo"):
    if os.path.isdir(_p) and _p not in sys.path:
        sys.path.insert(0, _p)

import numpy as np
import ml_dtypes

import concourse.bass as bass
import concourse.mybir as mybir
import concourse.tile as tile
from concourse import bacc
from concourse.bass_utils import run_bass_kernel_spmd

F32 = mybir.dt.float32
BF = mybir.dt.bfloat16

B, T, C = 2, 2048, 1024
NHEAD_TOT = 16
DH = 64
NCORES = 8
NH = 4           # heads per core
NPAIR = 2        # head pairs per core
CK = C // 128    # contraction chunks (8)
TT = 512         # t-tile width
NTT = T // TT    # 4
FQK = 2 * NH * DH   # 512 cols of qkv^T for q+k
FV = NH * DH        # 256 cols for v
EXPF = mybir.ActivationFunctionType.Exp


def build_nc():
    nc = bacc.Bacc("TRN2", target_bir_lowering=False, debug=False)

    xT = nc.dram_tensor("xT", [C, T], BF, kind="ExternalInput")
    wqkvT = nc.dram_tensor("wqkvT", [C, FQK + FV], BF, kind="ExternalInput")
    woutT = nc.dram_tensor("woutT", [NH * DH, C], BF, kind="ExternalInput")
    y = nc.dram_tensor("y", [T, C], BF, kind="ExternalOutput")

    with tile.TileContext(nc) as tc:
        with (
            tc.tile_pool(name="const", bufs=1) as const,
            tc.tile_pool(name="ptp", bufs=8) as ptp,
            tc.tile_pool(name="rcp", bufs=4) as rcp,
            tc.tile_pool(name="rbp", bufs=4) as rbp,
            tc.tile_pool(name="yp", bufs=3) as yp,
            tc.tile_pool(name="psA", bufs=3, space="PSUM") as psA,
            tc.tile_pool(name="psV", bufs=2, space="PSUM") as psV,
        ):
            # ---- persistent SBUF ----
            wqkv_sb = const.tile([128, CK, FQK + FV], BF)
            wout_sb = const.tile([128, NPAIR, C], BF)
            xt = [const.tile([128, CK, TT], BF, name=f"xt{t}") for t in range(NTT)]
            # per-tt q/k: [128 rows = hi*64+d, 4 = (q pr0, q pr1, k pr0, k pr1), TT]
            qk = [const.tile([128, 4, TT], BF, name=f"qk{t}") for t in range(NTT)]
            # per-tt V (s-major) + ones column: [128 s, 4 si, NH heads, DH+1]
            vt = [const.tile([128, 4, NH, DH + 1], BF, name=f"vt{t}") for t in range(NTT)]
            # per-tt normalized O^T: [128 rows = hi*64+d, pr, TT]
            oT = [const.tile([128, NPAIR, TT], BF, name=f"oT{t}") for t in range(NTT)]

            # causal mask for diagonal quads: m0[p, i, t] = 1 if t >= 128*i + p
            m0 = const.tile([128, 2, TT], BF)

            nc.sync.dma_start(
                wqkv_sb, wqkvT.rearrange("(ck p) f -> p ck f", p=128))
            nc.sync.dma_start(
                wout_sb, woutT.rearrange("(pr p) c -> p pr c", p=128))
            for t in range(NTT):
                nc.sync.dma_start(
                    xt[t], xT.rearrange("(ck p) t -> p ck t", p=128)
                    [:, :, t * TT:(t + 1) * TT])
                # 1.0 in bf16 for the softmax-sum ones column
                nc.vector.memset(
                    vt[t][:, :, :, DH:DH + 1].bitcast(mybir.dt.uint16), 0x3F80)
            nc.vector.memset(m0.bitcast(mybir.dt.uint16), 0x3F80)
            nc.gpsimd.affine_select(
                out=m0, in_=m0, compare_op=mybir.AluOpType.is_ge,
                fill=0.0, base=0, channel_multiplier=-1,
                pattern=[[-128, 2], [1, TT]],
            )

            def proj_qk(tt, fq):
                """fq=0: q pairs -> qk[tt][:, 0:2]; fq=1: k pairs -> [:, 2:4]."""
                ps = psA.tile([128, 2, TT], F32, tag="ps")
                for half in range(2):
                    ft = fq * 2 + half
                    for ci in range(CK):
                        nc.tensor.matmul(
                            ps[:, half, :],
                            wqkv_sb[:, ci, ft * 128:(ft + 1) * 128],
                            xt[tt][:, ci, :],
                            start=(ci == 0), stop=(ci == CK - 1),
                        )
                nc.vector.tensor_copy(qk[tt][:, fq * 2:fq * 2 + 2, :], ps)

            def proj_v(tt):
                for vg in range(2):
                    ps = psA.tile([128, 2, TT], F32, tag="ps")
                    for half in range(2):
                        off = (vg * 2 + half) * 128
                        for ci in range(CK):
                            nc.tensor.matmul(
                                ps[:, half, 0:FV],
                                xt[tt][:, ci, off:off + 128],
                                wqkv_sb[:, ci, FQK:FQK + FV],
                                start=(ci == 0), stop=(ci == CK - 1),
                            )
                    nc.vector.tensor_copy(
                        vt[tt][:, vg * 2:vg * 2 + 2, :, 0:DH],
                        ps[:, :, 0:FV].rearrange("p s (h d) -> p s h d", h=NH),
                    )

            def attn_pairs(tt, pr, pv, sq_list):
                """QK -> exp -> (mask) -> PV for the given ss-pair indices.
                PV is deferred two sq-groups behind QK so the exp/mask chain
                never stalls the PE."""
                groups = []

                def flush_one():
                    for (hi, ss0, pt, width, toff) in groups.pop(0):
                        for i in range(2):
                            ss = ss0 + i
                            nc.tensor.matmul(
                                pv[hi][:, toff:toff + width],
                                vt[ss // 4][:, ss % 4, pr * 2 + hi, :],
                                pt[:, i, 0:width],
                                start=(ss == 0), stop=(ss == 4 * tt + 3),
                            )

                for sq in sq_list:
                    ss0 = 2 * sq
                    pair1 = (ss0 == 4 * tt + 2)   # upper diagonal pair
                    width = TT // 2 if pair1 else TT
                    toff = TT // 2 if pair1 else 0
                    new = []
                    for hi in range(2):
                        ps = psA.tile([128, 2, TT], F32, tag="ps")
                        for i in range(2):
                            ss = ss0 + i
                            nc.tensor.matmul(
                                ps[:, i, 0:width],
                                qk[ss // 4][hi * 64:(hi + 1) * 64, 2 + pr,
                                            (ss % 4) * 128:(ss % 4) * 128 + 128],
                                qk[tt][hi * 64:(hi + 1) * 64, pr,
                                       toff:toff + width],
                            )
                        pt = ptp.tile([128, 2, TT], BF, tag="pt")
                        nc.scalar.activation(
                            pt[:, :, 0:width], ps[:, :, 0:width], EXPF,
                            scale=0.125)
                        if ss0 >= 4 * tt:   # diagonal quad: zero where s > t
                            nc.vector.tensor_mul(
                                pt[:, :, 0:width], pt[:, :, 0:width],
                                m0[:, :, 0:width])
                        new.append((hi, ss0, pt, width, toff))
                    groups.append(new)
                    if len(groups) > 2:
                        flush_one()
                while groups:
                    flush_one()

            def norm(tt, pr, pv):
                """oT[tt][:, pr] = pv_rows / L via recip + partition bcast."""
                rrows = []
                for hi in range(2):
                    rrow = rcp.tile([1, TT], F32, tag="rrow")
                    nc.vector.reciprocal(rrow, pv[hi][DH:DH + 1, :])
                    rrows.append(rrow)
                rbs = []
                for hi in range(2):
                    rb = rbp.tile([64, TT], F32, tag="rb")
                    nc.gpsimd.partition_broadcast(rb, rrows[hi][0:1, :],
                                                  channels=64)
                    rbs.append(rb)
                for hi in range(2):
                    nc.vector.tensor_mul(
                        oT[tt][hi * 64:(hi + 1) * 64, pr, :],
                        pv[hi][0:DH, :],
                        rbs[hi],
                    )

            def out_proj(tt):
                for tq4 in range(4):
                    tq = tt * 4 + tq4
                    ps = psA.tile([128, 2, TT], F32, tag="ps")
                    for ot in range(2):
                        for pr in range(NPAIR):
                            nc.tensor.matmul(
                                ps[:, ot, :],
                                oT[tt][:, pr, tq4 * 128:(tq4 + 1) * 128],
                                wout_sb[:, pr, ot * TT:(ot + 1) * TT],
                                start=(pr == 0), stop=(pr == NPAIR - 1),
                            )
                    yt = yp.tile([128, C], BF, tag="yt")
                    nc.vector.tensor_copy(yt, ps)
                    nc.sync.dma_start(y[tq * 128:(tq + 1) * 128, :], yt)

            # ---- pipelined main loop ----
            # out_proj(tt) is deferred one t-tile so the PE has queued work
            # while the norm chain (recip->bcast->mul) completes.
            for tt in range(NTT):
                n_sq = 2 * (tt + 1)
                nondiag = list(range(n_sq - 2))
                diag = [n_sq - 2, n_sq - 1]
                proj_qk(tt, 0)                      # q(tt)
                pv0 = [psV.tile([DH + 1, TT], F32, tag="pv",
                                name=f"pv{tt}_0_{k}") for k in range(2)]
                attn_pairs(tt, 0, pv0, nondiag)     # pr=0 vs old k/v
                proj_qk(tt, 1)                      # k(tt)
                proj_v(tt)                          # v(tt)
                if tt > 0:
                    out_proj(tt - 1)
                attn_pairs(tt, 0, pv0, diag)
                norm(tt, 0, pv0)
                pv1 = [psV.tile([DH + 1, TT], F32, tag="pv",
                                name=f"pv{tt}_1_{k}") for k in range(2)]
                attn_pairs(tt, 1, pv1, nondiag + diag)
                norm(tt, 1, pv1)
            out_proj(NTT - 1)

    nc.compile()
    return nc


_NC_CACHE = None


def _get_nc():
    global _NC_CACHE
    if _NC_CACHE is None:
        _NC_CACHE = build_nc()
    return _NC_CACHE


def make_in_maps(x, W_qkv, W_out):
    bf16 = ml_dtypes.bfloat16
    x = np.asarray(x, dtype=np.float32)
    W_qkv = np.asarray(W_qkv, dtype=np.float32)
    W_out = np.asarray(W_out, dtype=np.float32)
    xTb = [np.ascontiguousarray(x[b].T.astype(bf16)) for b in range(B)]
    in_maps = []
    for c in range(NCORES):
        b, g = c // 4, c % 4
        rq = W_qkv[g * 256:(g + 1) * 256]                   # q rows, heads 4g..4g+3
        rk = W_qkv[C + g * 256:C + (g + 1) * 256]           # k rows
        rv = W_qkv[2 * C + g * 256:2 * C + (g + 1) * 256]   # v rows
        wq = np.ascontiguousarray(
            np.concatenate([rq, rk, rv], axis=0).T.astype(bf16))
        wo = np.ascontiguousarray(W_out[:, g * 256:(g + 1) * 256].T.astype(bf16))
        in_maps.append({"xT": xTb[b], "wqkvT": wq, "woutT": wo})
    return in_maps


def kernel(x, W_qkv, W_out):
    nc = _get_nc()
    in_maps = make_in_maps(x, W_qkv, W_out)
    res = run_bass_kernel_spmd(nc, in_maps, core_ids=list(range(NCORES)))
    kernel.last_results = res
    y = np.zeros((B, T, C), dtype=np.float32)
    for c in range(NCORES):
        y[c // 4] += np.asarray(res.results[c]["y"], dtype=np.float32)
    return y


# revision 9
# speedup vs baseline: 1.1931x; 1.1931x over previous
"""Causal self-attention (B=2, T=2048, D=1024, H=16) on 8 trn2 cores.

Sharding: tensor-parallel over heads x data-parallel over batch.
Core c handles batch b = c // 4, head group g = c % 4 (heads 4g..4g+3).
Host pre-slices/pre-transposes weight+activation shards (cast to bf16);
each core returns a partial y (its heads' contribution); host sums
groups of 4.

Kernel structure (v2): pipelined per t-tile of 512 —
  q-proj(tt) -> attention on old k/v s-chunks -> k/v-proj(tt) ->
  diagonal-quad attention -> on-chip softmax norm -> out-proj(tt).
All matmuls bf16 (f32 psum). Softmax sums come from a ones-column in V;
normalization is reciprocal (DVE) + partition_broadcast (Pool) + mul,
no DRAM roundtrip. Upper half of each diagonal quad is computed at half
t-width.
"""

import os
import sys

for _p in ("/opt/trn_rl_repo", "/root/.axon_site/_ro/trn_rl_repo"):
    if os.path.isdir(_p) and _p not in sys.path:
        sys.path.insert(0, _p)

import numpy as np
import ml_dtypes

import concourse.bass as bass
import concourse.mybir as mybir
import concourse.tile as tile
from concourse import bacc
from concourse.bass_utils import run_bass_kernel_spmd

F32 = mybir.dt.float32
BF = mybir.dt.bfloat16

B, T, C = 2, 2048, 1024
NHEAD_TOT = 16
DH = 64
NCORES = 8
NH = 4           # heads per core
NPAIR = 2        # head pairs per core
CK = C // 128    # contraction chunks (8)
TT = 512         # t-tile width
NTT = T // TT    # 4
FQK = 2 * NH * DH   # 512 cols of qkv^T for q+k
FV = NH * DH        # 256 cols for v
EXPF = mybir.ActivationFunctionType.Exp


def build_nc():
    nc = bacc.Bacc("TRN2", target_bir_lowering=False, debug=False)

    xT = nc.dram_tensor("xT", [C, T], BF, kind="ExternalInput")
    wqkvT = nc.dram_tensor("wqkvT", [C, FQK + FV], BF, kind="ExternalInput")
    woutT = nc.dram_tensor("woutT", [NH * DH, C], BF, kind="ExternalInput")
    y = nc.dram_tensor("y", [T, C], BF, kind="ExternalOutput")

    with tile.TileContext(nc) as tc:
        with (
            tc.tile_pool(name="const", bufs=1) as const,
            tc.tile_pool(name="ptp", bufs=8) as ptp,
            tc.tile_pool(name="rcp", bufs=4) as rcp,
            tc.tile_pool(name="rbp", bufs=4) as rbp,
            tc.tile_pool(name="yp", bufs=3) as yp,
            tc.tile_pool(name="psA", bufs=3, space="PSUM") as psA,
            tc.tile_pool(name="psV", bufs=2, space="PSUM") as psV,
        ):
            # ---- persistent SBUF ----
            wqkv_sb = const.tile([128, CK, FQK + FV], BF)
            wout_sb = const.tile([128, NPAIR, C], BF)
            xt = [const.tile([128, CK, TT], BF, name=f"xt{t}") for t in range(NTT)]
            # per-tt q/k: [128 rows = hi*64+d, 4 = (q pr0, q pr1, k pr0, k pr1), TT]
            qk = [const.tile([128, 4, TT], BF, name=f"qk{t}") for t in range(NTT)]
            # per-tt V (s-major) + ones column: [128 s, 4 si, NH heads, DH+1]
            vt = [const.tile([128, 4, NH, DH + 1], BF, name=f"vt{t}") for t in range(NTT)]
            # per-tt normalized O^T: [128 rows = hi*64+d, pr, TT]
            oT = [const.tile([128, NPAIR, TT], BF, name=f"oT{t}") for t in range(NTT)]

            # causal mask for diagonal quads: m0[p, i, t] = 1 if t >= 128*i + p
            m0 = const.tile([128, 2, TT], BF)

            nc.sync.dma_start(
                wqkv_sb, wqkvT.rearrange("(ck p) f -> p ck f", p=128))
            nc.sync.dma_start(
                wout_sb, woutT.rearrange("(pr p) c -> p pr c", p=128))
            for t in range(NTT):
                nc.sync.dma_start(
                    xt[t], xT.rearrange("(ck p) t -> p ck t", p=128)
                    [:, :, t * TT:(t + 1) * TT])
                # 1.0 in bf16 for the softmax-sum ones column
                nc.vector.memset(
                    vt[t][:, :, :, DH:DH + 1].bitcast(mybir.dt.uint16), 0x3F80)
            nc.vector.memset(m0.bitcast(mybir.dt.uint16), 0x3F80)
            nc.gpsimd.affine_select(
                out=m0, in_=m0, compare_op=mybir.AluOpType.is_ge,
                fill=0.0, base=0, channel_multiplier=-1,
                pattern=[[-128, 2], [1, TT]],
            )

            def proj_qk(tt, fq):
                """fq=0: q pairs -> qk[tt][:, 0:2]; fq=1: k pairs -> [:, 2:4]."""
                ps = psA.tile([128, 2, TT], F32, tag="ps")
                for half in range(2):
                    ft = fq * 2 + half
                    for ci in range(CK):
                        nc.tensor.matmul(
                            ps[:, half, :],
                            wqkv_sb[:, ci, ft * 128:(ft + 1) * 128],
                            xt[tt][:, ci, :],
                            start=(ci == 0), stop=(ci == CK - 1),
                        )
                nc.vector.tensor_copy(qk[tt][:, fq * 2:fq * 2 + 2, :], ps)

            def proj_v(tt):
                for vg in range(2):
                    ps = psA.tile([128, 2, TT], F32, tag="ps")
                    for half in range(2):
                        off = (vg * 2 + half) * 128
                        for ci in range(CK):
                            nc.tensor.matmul(
                                ps[:, half, 0:FV],
                                xt[tt][:, ci, off:off + 128],
                                wqkv_sb[:, ci, FQK:FQK + FV],
                                start=(ci == 0), stop=(ci == CK - 1),
                            )
                    nc.vector.tensor_copy(
                        vt[tt][:, vg * 2:vg * 2 + 2, :, 0:DH],
                        ps[:, :, 0:FV].rearrange("p s (h d) -> p s h d", h=NH),
                    )

            def attn_pairs(tt, pr, pv, sq_list):
                """QK -> exp -> (mask) -> PV for the given ss-pair indices.
                PV is deferred two sq-groups behind QK so the exp/mask chain
                never stalls the PE."""
                groups = []

                def flush_one():
                    for (hi, ss0, pt, width, toff) in groups.pop(0):
                        for i in range(2):
                            ss = ss0 + i
                            nc.tensor.matmul(
                                pv[hi][:, toff:toff + width],
                                vt[ss // 4][:, ss % 4, pr * 2 + hi, :],
                                pt[:, i, 0:width],
                                start=(ss == 0), stop=(ss == 4 * tt + 3),
                            )

                for sq in sq_list:
                    ss0 = 2 * sq
                    pair1 = (ss0 == 4 * tt + 2)   # upper diagonal pair
                    width = TT // 2 if pair1 else TT
                    toff = TT // 2 if pair1 else 0
                    new = []
                    for hi in range(2):
                        ps = psA.tile([128, 2, TT], F32, tag="ps")
                        for i in range(2):
                            ss = ss0 + i
                            nc.tensor.matmul(
                                ps[:, i, 0:width],
                                qk[ss // 4][hi * 64:(hi + 1) * 64, 2 + pr,
                                            (ss % 4) * 128:(ss % 4) * 128 + 128],
                                qk[tt][hi * 64:(hi + 1) * 64, pr,
                                       toff:toff + width],
                            )
                        pt = ptp.tile([128, 2, TT], BF, tag="pt")
                        nc.scalar.activation(
                            pt[:, :, 0:width], ps[:, :, 0:width], EXPF,
                            scale=0.125)
                        if ss0 >= 4 * tt:   # diagonal quad: zero where s > t
                            nc.vector.tensor_mul(
                                pt[:, :, 0:width], pt[:, :, 0:width],
                                m0[:, :, 0:width])
                        new.append((hi, ss0, pt, width, toff))
                    groups.append(new)
                    if len(groups) > 2:
                        flush_one()
                while groups:
                    flush_one()

            def norm(tt, pr, pv):
                """oT[tt][:, pr] = pv_rows / L via recip + partition bcast."""
                rrows = []
                for hi in range(2):
                    lrow = rcp.tile([1, TT], F32, tag="rrow")
                    nc.vector.tensor_copy(lrow, pv[hi][DH:DH + 1, :])
                    rrow = rcp.tile([1, TT], F32, tag="rrow")
                    nc.vector.reciprocal_approx_fast(rrow, lrow)
                    rrows.append(rrow)
                rbs = []
                for hi in range(2):
                    rb = rbp.tile([64, TT], F32, tag="rb")
                    nc.gpsimd.partition_broadcast(rb, rrows[hi][0:1, :],
                                                  channels=64)
                    rbs.append(rb)
                for hi in range(2):
                    nc.vector.tensor_mul(
                        oT[tt][hi * 64:(hi + 1) * 64, pr, :],
                        pv[hi][0:DH, :],
                        rbs[hi],
                    )

            def out_proj(tt):
                for tq4 in range(4):
                    tq = tt * 4 + tq4
                    ps = psA.tile([128, 2, TT], F32, tag="ps")
                    for ot in range(2):
                        for pr in range(NPAIR):
                            nc.tensor.matmul(
                                ps[:, ot, :],
                                oT[tt][:, pr, tq4 * 128:(tq4 + 1) * 128],
                                wout_sb[:, pr, ot * TT:(ot + 1) * TT],
                                start=(pr == 0), stop=(pr == NPAIR - 1),
                            )
                    yt = yp.tile([128, C], BF, tag="yt")
                    nc.vector.tensor_copy(yt, ps)
                    nc.sync.dma_start(y[tq * 128:(tq + 1) * 128, :], yt)

            # ---- pipelined main loop ----
            # out_proj(tt) is deferred one t-tile so the PE has queued work
            # while the norm chain (recip->bcast->mul) completes.
            for tt in range(NTT):
                n_sq = 2 * (tt + 1)
                nondiag = list(range(n_sq - 2))
                diag = [n_sq - 2, n_sq - 1]
                proj_qk(tt, 0)                      # q(tt)
                pv0 = [psV.tile([DH + 1, TT], F32, tag="pv",
                                name=f"pv{tt}_0_{k}") for k in range(2)]
                attn_pairs(tt, 0, pv0, nondiag)     # pr=0 vs old k/v
                proj_qk(tt, 1)                      # k(tt)
                proj_v(tt)                          # v(tt)
                if tt > 0:
                    out_proj(tt - 1)
                attn_pairs(tt, 0, pv0, diag)
                norm(tt, 0, pv0)
                pv1 = [psV.tile([DH + 1, TT], F32, tag="pv",
                                name=f"pv{tt}_1_{k}") for k in range(2)]
                attn_pairs(tt, 1, pv1, nondiag + diag)
                norm(tt, 1, pv1)
            out_proj(NTT - 1)

    nc.compile()
    return nc


_NC_CACHE = None


def _get_nc():
    global _NC_CACHE
    if _NC_CACHE is None:
        _NC_CACHE = build_nc()
    return _NC_CACHE


def make_in_maps(x, W_qkv, W_out):
    bf16 = ml_dtypes.bfloat16
    x = np.asarray(x, dtype=np.float32)
    W_qkv = np.asarray(W_qkv, dtype=np.float32)
    W_out = np.asarray(W_out, dtype=np.float32)
    xTb = [np.ascontiguousarray(x[b].T.astype(bf16)) for b in range(B)]
    in_maps = []
    for c in range(NCORES):
        b, g = c // 4, c % 4
        rq = W_qkv[g * 256:(g + 1) * 256]                   # q rows, heads 4g..4g+3
        rk = W_qkv[C + g * 256:C + (g + 1) * 256]           # k rows
        rv = W_qkv[2 * C + g * 256:2 * C + (g + 1) * 256]   # v rows
        wq = np.ascontiguousarray(
            np.concatenate([rq, rk, rv], axis=0).T.astype(bf16))
        wo = np.ascontiguousarray(W_out[:, g * 256:(g + 1) * 256].T.astype(bf16))
        in_maps.append({"xT": xTb[b], "wqkvT": wq, "woutT": wo})
    return in_maps


def kernel(x, W_qkv, W_out):
    nc = _get_nc()
    in_maps = make_in_maps(x, W_qkv, W_out)
    res = run_bass_kernel_spmd(nc, in_maps, core_ids=list(range(NCORES)))
    kernel.last_results = res
    y = np.zeros((B, T, C), dtype=np.float32)
    for c in range(NCORES):
        y[c // 4] += np.asarray(res.results[c]["y"], dtype=np.float32)
    return y


# revision 10
# speedup vs baseline: 1.2282x; 1.0294x over previous
"""Causal self-attention (B=2, T=2048, D=1024, H=16) on 8 trn2 cores.

Sharding: tensor-parallel over heads x data-parallel over batch.
Core c handles batch b = c // 4, head group g = c % 4 (heads 4g..4g+3).
Host pre-slices/pre-transposes weight+activation shards (cast to bf16);
each core returns a partial y (its heads' contribution); host sums
groups of 4.

Kernel structure (v2): pipelined per t-tile of 512 —
  q-proj(tt) -> attention on old k/v s-chunks -> k/v-proj(tt) ->
  diagonal-quad attention -> on-chip softmax norm -> out-proj(tt).
All matmuls bf16 (f32 psum). Softmax sums come from a ones-column in V;
normalization is reciprocal (DVE) + partition_broadcast (Pool) + mul,
no DRAM roundtrip. Upper half of each diagonal quad is computed at half
t-width.
"""

import os
import sys

for _p in ("/opt/trn_rl_repo", "/root/.axon_site/_ro/trn_rl_repo"):
    if os.path.isdir(_p) and _p not in sys.path:
        sys.path.insert(0, _p)

import numpy as np
import ml_dtypes

import concourse.bass as bass
import concourse.mybir as mybir
import concourse.tile as tile
from concourse import bacc
from concourse.bass_utils import run_bass_kernel_spmd

F32 = mybir.dt.float32
BF = mybir.dt.bfloat16

B, T, C = 2, 2048, 1024
NHEAD_TOT = 16
DH = 64
NCORES = 8
NH = 4           # heads per core
NPAIR = 2        # head pairs per core
CK = C // 128    # contraction chunks (8)
TT = 512         # t-tile width
NTT = T // TT    # 4
FQK = 2 * NH * DH   # 512 cols of qkv^T for q+k
FV = NH * DH        # 256 cols for v
EXPF = mybir.ActivationFunctionType.Exp


def build_nc():
    nc = bacc.Bacc("TRN2", target_bir_lowering=False, debug=False)

    xT = nc.dram_tensor("xT", [C, T], BF, kind="ExternalInput")
    wqkvT = nc.dram_tensor("wqkvT", [C, FQK + FV], BF, kind="ExternalInput")
    woutT = nc.dram_tensor("woutT", [NH * DH, C], BF, kind="ExternalInput")
    y = nc.dram_tensor("y", [T, C], BF, kind="ExternalOutput")

    with tile.TileContext(nc) as tc:
        with (
            tc.tile_pool(name="const", bufs=1) as const,
            tc.tile_pool(name="ptp", bufs=8) as ptp,
            tc.tile_pool(name="rcp", bufs=4) as rcp,
            tc.tile_pool(name="rbp", bufs=4) as rbp,
            tc.tile_pool(name="yp", bufs=3) as yp,
            tc.tile_pool(name="psA", bufs=3, space="PSUM") as psA,
            tc.tile_pool(name="psV", bufs=2, space="PSUM") as psV,
        ):
            # ---- persistent SBUF ----
            wqkv_sb = const.tile([128, CK, FQK + FV], BF)
            wout_sb = const.tile([128, NPAIR, C], BF)
            xt = [const.tile([128, CK, TT], BF, name=f"xt{t}") for t in range(NTT)]
            # per-tt q/k: [128 rows = hi*64+d, 4 = (q pr0, q pr1, k pr0, k pr1), TT]
            qk = [const.tile([128, 4, TT], BF, name=f"qk{t}") for t in range(NTT)]
            # per-tt V (s-major) + ones column: [128 s, 4 si, NH heads, DH+1]
            vt = [const.tile([128, 4, NH, DH + 1], BF, name=f"vt{t}") for t in range(NTT)]
            # per-tt normalized O^T: [128 rows = hi*64+d, pr, TT]
            oT = [const.tile([128, NPAIR, TT], BF, name=f"oT{t}") for t in range(NTT)]

            # causal mask for diagonal quads: m0[p, i, t] = 1 if t >= 128*i + p
            m0 = const.tile([128, 2, TT], BF)

            # critical-path first: the halves of wqkv/x(t=0) that feed the
            # first projection matmuls, then everything else.
            wq_src = wqkvT.rearrange("(ck p) f -> p ck f", p=128)
            x_src = xT.rearrange("(ck p) t -> p ck t", p=128)
            nc.sync.dma_start(wqkv_sb[:, 0:4, :], wq_src[:, 0:4, :])
            nc.sync.dma_start(xt[0][:, 0:4, :], x_src[:, 0:4, 0:TT])
            nc.sync.dma_start(wqkv_sb[:, 4:8, :], wq_src[:, 4:8, :])
            nc.sync.dma_start(xt[0][:, 4:8, :], x_src[:, 4:8, 0:TT])
            for t in range(1, NTT):
                nc.sync.dma_start(xt[t], x_src[:, :, t * TT:(t + 1) * TT])
            nc.sync.dma_start(
                wout_sb, woutT.rearrange("(pr p) c -> p pr c", p=128))
            for t in range(NTT):
                # 1.0 in bf16 for the softmax-sum ones column
                nc.vector.memset(
                    vt[t][:, :, :, DH:DH + 1].bitcast(mybir.dt.uint16), 0x3F80)
            nc.vector.memset(m0.bitcast(mybir.dt.uint16), 0x3F80)
            nc.gpsimd.affine_select(
                out=m0, in_=m0, compare_op=mybir.AluOpType.is_ge,
                fill=0.0, base=0, channel_multiplier=-1,
                pattern=[[-128, 2], [1, TT]],
            )

            def proj_qk(tt, fq):
                """fq=0: q pairs -> qk[tt][:, 0:2]; fq=1: k pairs -> [:, 2:4]."""
                ps = psA.tile([128, 2, TT], F32, tag="ps")
                for half in range(2):
                    ft = fq * 2 + half
                    for ci in range(CK):
                        nc.tensor.matmul(
                            ps[:, half, :],
                            wqkv_sb[:, ci, ft * 128:(ft + 1) * 128],
                            xt[tt][:, ci, :],
                            start=(ci == 0), stop=(ci == CK - 1),
                        )
                nc.vector.tensor_copy(qk[tt][:, fq * 2:fq * 2 + 2, :], ps)

            def proj_v(tt):
                for vg in range(2):
                    ps = psA.tile([128, 2, TT], F32, tag="ps")
                    for half in range(2):
                        off = (vg * 2 + half) * 128
                        for ci in range(CK):
                            nc.tensor.matmul(
                                ps[:, half, 0:FV],
                                xt[tt][:, ci, off:off + 128],
                                wqkv_sb[:, ci, FQK:FQK + FV],
                                start=(ci == 0), stop=(ci == CK - 1),
                            )
                    nc.vector.tensor_copy(
                        vt[tt][:, vg * 2:vg * 2 + 2, :, 0:DH],
                        ps[:, :, 0:FV].rearrange("p s (h d) -> p s h d", h=NH),
                    )

            def attn_pairs(tt, pr, pv, sq_list):
                """QK -> exp -> (mask) -> PV for the given ss-pair indices.
                PV is deferred two sq-groups behind QK so the exp/mask chain
                never stalls the PE."""
                groups = []

                def flush_one():
                    for (hi, ss0, pt, width, toff) in groups.pop(0):
                        for i in range(2):
                            ss = ss0 + i
                            nc.tensor.matmul(
                                pv[hi][:, toff:toff + width],
                                vt[ss // 4][:, ss % 4, pr * 2 + hi, :],
                                pt[:, i, 0:width],
                                start=(ss == 0), stop=(ss == 4 * tt + 3),
                            )

                for sq in sq_list:
                    ss0 = 2 * sq
                    pair1 = (ss0 == 4 * tt + 2)   # upper diagonal pair
                    width = TT // 2 if pair1 else TT
                    toff = TT // 2 if pair1 else 0
                    new = []
                    for hi in range(2):
                        ps = psA.tile([128, 2, TT], F32, tag="ps")
                        for i in range(2):
                            ss = ss0 + i
                            nc.tensor.matmul(
                                ps[:, i, 0:width],
                                qk[ss // 4][hi * 64:(hi + 1) * 64, 2 + pr,
                                            (ss % 4) * 128:(ss % 4) * 128 + 128],
                                qk[tt][hi * 64:(hi + 1) * 64, pr,
                                       toff:toff + width],
                            )
                        pt = ptp.tile([128, 2, TT], BF, tag="pt")
                        nc.scalar.activation(
                            pt[:, :, 0:width], ps[:, :, 0:width], EXPF,
                            scale=0.125)
                        if ss0 >= 4 * tt:   # diagonal quad: zero where s > t
                            nc.vector.tensor_mul(
                                pt[:, :, 0:width], pt[:, :, 0:width],
                                m0[:, :, 0:width])
                        new.append((hi, ss0, pt, width, toff))
                    groups.append(new)
                    if len(groups) > 2:
                        flush_one()
                while groups:
                    flush_one()

            def norm(tt, pr, pv):
                """oT[tt][:, pr] = pv_rows / L via recip + partition bcast."""
                rrows = []
                for hi in range(2):
                    lrow = rcp.tile([1, TT], F32, tag="rrow")
                    nc.vector.tensor_copy(lrow, pv[hi][DH:DH + 1, :])
                    rrow = rcp.tile([1, TT], F32, tag="rrow")
                    nc.vector.reciprocal_approx_fast(rrow, lrow)
                    rrows.append(rrow)
                rbs = []
                for hi in range(2):
                    rb = rbp.tile([64, TT], F32, tag="rb")
                    nc.gpsimd.partition_broadcast(rb, rrows[hi][0:1, :],
                                                  channels=64)
                    rbs.append(rb)
                for hi in range(2):
                    nc.vector.tensor_mul(
                        oT[tt][hi * 64:(hi + 1) * 64, pr, :],
                        pv[hi][0:DH, :],
                        rbs[hi],
                    )

            def out_proj(tt):
                for tq4 in range(4):
                    tq = tt * 4 + tq4
                    ps = psA.tile([128, 2, TT], F32, tag="ps")
                    for ot in range(2):
                        for pr in range(NPAIR):
                            nc.tensor.matmul(
                                ps[:, ot, :],
                                oT[tt][:, pr, tq4 * 128:(tq4 + 1) * 128],
                                wout_sb[:, pr, ot * TT:(ot + 1) * TT],
                                start=(pr == 0), stop=(pr == NPAIR - 1),
                            )
                    yt = yp.tile([128, C], BF, tag="yt")
                    nc.vector.tensor_copy(yt, ps)
                    nc.sync.dma_start(y[tq * 128:(tq + 1) * 128, :], yt)

            # ---- pipelined main loop ----
            # out_proj(tt) is deferred one t-tile so the PE has queued work
            # while the norm chain (recip->bcast->mul) completes.
            for tt in range(NTT):
                n_sq = 2 * (tt + 1)
                nondiag = list(range(n_sq - 2))
                diag = [n_sq - 2, n_sq - 1]
                proj_qk(tt, 0)                      # q(tt)
                pv0 = [psV.tile([DH + 1, TT], F32, tag="pv",
                                name=f"pv{tt}_0_{k}") for k in range(2)]
                attn_pairs(tt, 0, pv0, nondiag)     # pr=0 vs old k/v
                proj_qk(tt, 1)                      # k(tt)
                proj_v(tt)                          # v(tt)
                if tt > 0:
                    out_proj(tt - 1)
                attn_pairs(tt, 0, pv0, diag)
                norm(tt, 0, pv0)
                pv1 = [psV.tile([DH + 1, TT], F32, tag="pv",
                                name=f"pv{tt}_1_{k}") for k in range(2)]
                attn_pairs(tt, 1, pv1, nondiag + diag)
                norm(tt, 1, pv1)
            out_proj(NTT - 1)

    nc.compile()
    return nc


_NC_CACHE = None


def _get_nc():
    global _NC_CACHE
    if _NC_CACHE is None:
        _NC_CACHE = build_nc()
    return _NC_CACHE


def make_in_maps(x, W_qkv, W_out):
    bf16 = ml_dtypes.bfloat16
    x = np.asarray(x, dtype=np.float32)
    W_qkv = np.asarray(W_qkv, dtype=np.float32)
    W_out = np.asarray(W_out, dtype=np.float32)
    xTb = [np.ascontiguousarray(x[b].T.astype(bf16)) for b in range(B)]
    in_maps = []
    for c in range(NCORES):
        b, g = c // 4, c % 4
        rq = W_qkv[g * 256:(g + 1) * 256]                   # q rows, heads 4g..4g+3
        rk = W_qkv[C + g * 256:C + (g + 1) * 256]           # k rows
        rv = W_qkv[2 * C + g * 256:2 * C + (g + 1) * 256]   # v rows
        wq = np.ascontiguousarray(
            np.concatenate([rq, rk, rv], axis=0).T.astype(bf16))
        wo = np.ascontiguousarray(W_out[:, g * 256:(g + 1) * 256].T.astype(bf16))
        in_maps.append({"xT": xTb[b], "wqkvT": wq, "woutT": wo})
    return in_maps


def kernel(x, W_qkv, W_out):
    nc = _get_nc()
    in_maps = make_in_maps(x, W_qkv, W_out)
    res = run_bass_kernel_spmd(nc, in_maps, core_ids=list(range(NCORES)))
    kernel.last_results = res
    y = np.zeros((B, T, C), dtype=np.float32)
    for c in range(NCORES):
        y[c // 4] += np.asarray(res.results[c]["y"], dtype=np.float32)
    return y
